# revision 22
# baseline (speedup 1.0000x reference)
"""Trainium2 Bass kernel for nn_CombinedGraphLoss (graph-loss over 8192x8192 adj).

loss = sum((A - decay)^2) + 0.1*sum|A - mean4(A)| + 0.001*sum(A^2)
with A = D^-1/2 relu(adj) D^-1/2, decay = exp(-0.1|i-j|).

Strategy (8 cores, row-sharded, full inputs per core):
  - each core gets relu(its 1024-row shard + 1 halo row each side) pre-converted
    to bf16 on the host; both passes stream the same 9 overlapping 128-row
    tiles (stride 126) from HBM.
  - pass1: row sums d, split ACT Copy(accum_out) / DVE reduce halves.
  - AllGather d -> global column factors colfac = exp(-0.5*ln(d+eps)) (bf16).
  - pass2 computes COLUMN-scaled A1c = adj*colfac only (one DVE op); the row
    factor dinv_i is folded into the stencil lhsT matrices on device and into
    the per-row partial sums on the host (f64):
      * stencil t = A - 0.25*(up+down+left+right) via 2 matmuls per 512-col
        chunk: tridiagonal Mv*dinv_i for center+vertical, diagonal
        NI*dinv_i = -0.25*dinv_i*I applied to lr = shift_left(A1c)+
        shift_right(A1c) (DVE); |t| row-sums via ACT Abs(accum_out) from PSUM.
      * sum A1c^2 row partials (DVE STT half / ACT Square half) -> host*dinv^2
      * band sum A1c*decay row partials (decay==0 in fp32 outside |i-j|>1039;
        2304-wide window, dynamic AP at pid-dependent offset) -> host*dinv
      * row sums d returned -> host computes dinv in f64.
  - decay terms decomposed: sum(A-decay)^2 = sumA^2 - 2*sum(A*decay) + sum(decay^2);
    sum decay^2 is analytic on host.
  - host applies row-ownership masks (overlap tiles) and reduces in float64.

All heavy elementwise work runs on DVE/ACT only -- GpSimd (Pool) executes no
big tensor ops (its software loops are ~25x slower and starve DVE through the
shared SBUF ports, which was the old kernel's bottleneck).

The wait-legalization passes below work around this toolchain's walrus, which
rejects instructions carrying more than one semaphore wait and miscompiles
EVENT_SEMAPHORE_RANGE_CLEAR.
"""

import numpy as np

import concourse.bass as bass
import concourse.mybir as mybir
from concourse import tile
from concourse.bass_utils import run_bass_kernel_spmd

from collections import defaultdict


def _facts_union(a, b):
    # facts: dict sem_id -> max value known reached
    for s, v in b.items():
        if a.get(s, -1) < v:
            a[s] = v
    return a


def strip_redundant_waits(nc, verbose=False):
    insts = []
    for bb in nc.m.functions[0].blocks:
        insts.extend(bb.instructions)

    # classify sems: updated by exactly one engine-proc (in-order) or not
    sem_updaters = defaultdict(set)
    for ins in insts:
        si = ins.sync_info
        if si is None:
            continue
        eng = getattr(ins, "engine", None)
        is_dma = type(ins).__name__ == "InstDMACopy"
        proc = ("dma", getattr(ins, "queue", "")) if is_dma else ("eng", str(eng))
        for u in si.on_update:
            sem_updaters[u.id].add(proc)
    inorder_sem = {
        s: next(iter(p))
        for s, p in sem_updaters.items()
        if len(p) == 1 and next(iter(p))[0] == "eng"
    }

    # walk in emission order, tracking per-proc facts and per-sem crossing facts
    proc_facts = defaultdict(dict)          # proc -> facts
    sem_cum = defaultdict(int)              # sem -> cumulative value
    sem_cross = defaultdict(list)           # sem -> [(cum_after, facts)]
    n_stripped = 0
    max_left = 0

    for ins in insts:
        si = ins.sync_info
        if si is None:
            continue
        eng = getattr(ins, "engine", None)
        is_dma = type(ins).__name__ == "InstDMACopy"
        proc = ("dma", getattr(ins, "queue", "")) if is_dma else ("eng", str(eng))
        in_order = not is_dma

        def wait_facts(w):
            # facts implied by "sem w.id >= w.value" holding
            f = {w.id: w.wait_value}
            if w.id in inorder_sem:
                for cum, facts in sem_cross[w.id]:
                    if cum >= w.wait_value:
                        _facts_union(f, facts)
                        break
            return f

        waits = list(si.on_wait)
        if len(waits) > 1:
            base = dict(proc_facts[proc]) if in_order else {}
            # engine-sem waits are always kept; other waits are dropped when
            # implied by program order + the kept engine-sem waits
            for w in waits:
                if w.id in inorder_sem:
                    _facts_union(base, wait_facts(w))
            keep = []
            drop = []
            for w in waits:
                if w.id not in inorder_sem and base.get(w.id, -1) >= w.wait_value:
                    drop.append(w)
                else:
                    keep.append(w)
            if drop:
                n_stripped += len(drop)
                from concourse import mybir

                ins.sync_info = mybir.SyncInfo(
                    on_wait=keep, on_update=list(si.on_update)
                )
                si = ins.sync_info
            waits = keep
        max_left = max(max_left, len(waits))

        # facts after this instruction completes
        myf = dict(proc_facts[proc]) if in_order else {}
        for w in waits:
            _facts_union(myf, wait_facts(w))
        for u in si.on_update:
            sem_cum[u.id] += u.update_value
            f = dict(myf)
            f[u.id] = sem_cum[u.id]
            sem_cross[u.id].append((sem_cum[u.id], f))
            if in_order:
                # own-sem value is part of this proc's program-order knowledge
                myf[u.id] = sem_cum[u.id]
        if in_order:
            proc_facts[proc] = myf

    if verbose:
        print(f"waitstrip: removed {n_stripped} waits, max remaining {max_left}")
    return n_stripped, max_left


def split_multi_waits(nc, verbose=False):
    """Rewrite instructions carrying >1 sync wait into a chain of same-engine
    NOPs each carrying one wait (in-order engine queues make this equivalent).
    Must run after strip_redundant_waits. DMACopy must already be single-wait.
    """
    from concourse import mybir

    n_split = 0
    for bb_w in nc.m.functions[0].blocks:
        il = bb_w.instructions
        i = 0
        while i < len(il):
            ins = il[i]
            si = ins.sync_info
            if si is not None and len(si.on_wait) > 1:
                # DMACopy here is SWDGE (engine=Pool): descriptor generation
                # runs in the Pool instruction stream, so a preceding Pool nop
                # legally gates it just like any compute instruction.
                waits = list(si.on_wait)
                extra, keep = waits[:-1], waits[-1:]
                for w in extra:
                    r = nc.engines[ins.engine].nop()
                    # pull the freshly appended nop out of whichever bb got it
                    nop_ins = r.ins
                    removed = False
                    for bb2 in nc.m.functions[0].blocks:
                        il2 = bb2.instructions
                        if il2 and il2[-1] is nop_ins:
                            il2.pop()
                            removed = True
                            break
                    assert removed, "could not locate appended nop"
                    nop_ins.sync_info = mybir.SyncInfo(on_wait=[w], on_update=[])
                    il.insert(i, nop_ins)
                    i += 1
                    n_split += 1
                ins.sync_info = mybir.SyncInfo(
                    on_wait=keep, on_update=list(si.on_update)
                )
            i += 1
    if verbose:
        print(f"waitstrip: split {n_split} waits onto nops")
    return n_split


def drop_broken_range_clear(nc, verbose=False):
    """This walrus snapshot miscompiles EVENT_SEMAPHORE_RANGE_CLEAR ("ISA
    wrong length"). It only matters for re-executing an already-loaded NEFF
    with dirty semaphores; drop it (verified empirically with back-to-back
    executions)."""
    n = 0
    for bb_w in nc.m.functions[0].blocks:
        il = bb_w.instructions
        for i in range(len(il) - 1, -1, -1):
            ins = il[i]
            if type(ins).__name__ == "InstISA" and getattr(ins, "isa_opcode", 0) == 176:
                del il[i]
                n += 1
    if verbose:
        print(f"waitstrip: dropped {n} EVENT_SEMAPHORE_RANGE_CLEAR")


def legalize_waits(nc, verbose=False):
    drop_broken_range_clear(nc, verbose=verbose)
    strip_redundant_waits(nc, verbose=verbose)
    split_multi_waits(nc, verbose=verbose)
    bad = []
    for bb_w in nc.m.functions[0].blocks:
        for ins in bb_w.instructions:
            si = ins.sync_info
            if si is not None and len(si.on_wait) > 1:
                bad.append(ins.name)
    assert not bad, f"instructions still multi-wait: {bad}"


N = 8192
NC = 8
SH = N // NC          # 1024 rows per core
LR = SH + 2           # local rows incl halos = 1026
ALPHA = 0.1
LAM = 0.1
GAMMA = 0.001

BW = 2304             # band width (covers |i-j| <= 1088 for every tile row)
PAD = 1152            # zero padding on each side of A1c_pad
APW = N + 2 * PAD     # 10496
CB = PAD              # first real column inside A1c_pad
R0S = [126 * k for k in range(8)] + [LR - 128]   # tile starts (local rows)
NT = len(R0S)

f32 = mybir.dt.float32
bf16 = mybir.dt.bfloat16
i32 = mybir.dt.int32
Alu = mybir.AluOpType
Act = mybir.ActivationFunctionType
X = mybir.AxisListType.X

# accumulator column layout in the [128, 96] f32 output
SM_COL = 0     # 36 cols: tile k quarter q -> 4k+q, rows 0..125 (no host scale)
A2A_COL = 36   # 9 cols: DVE half of sum A1c^2 (host * dinv^2)
A2B_COL = 45   # 9 cols: ACT half of sum A1c^2 (host * dinv^2)
BD_COL = 54    # 9 cols: band sum A1c*decay (host * dinv)
DA_COL = 63    # 9 cols: ACT half of row sums d
DB_COL = 72    # 9 cols: DVE half of row sums d
ACC_W = 96


def _build_nc():
    nc = bass.Bass(num_devices=NC)
    adj_in = nc.dram_tensor("adj_sh", [LR, N], bf16, kind="ExternalInput")
    res_out = nc.dram_tensor("res", [128, ACC_W], f32, kind="ExternalOutput")

    with tile.TileContext(nc) as tc:
        with (
            tc.tile_pool(name="const", bufs=1) as cp,
            tc.tile_pool(name="dram", bufs=1, space="DRAM") as dram,
            tc.tile_pool(name="io", bufs=3) as iop,
            tc.tile_pool(name="apad", bufs=1) as apadp,
            tc.tile_pool(name="lr", bufs=1) as lrp,
            tc.tile_pool(name="scr", bufs=1) as scrp,
            tc.tile_pool(name="ps", bufs=1, space="PSUM") as psp,
        ):
            acc = cp.tile([128, ACC_W], f32)
            nc.vector.memset(acc[:], 0.0)
            epsb = cp.tile([128, 1], f32)
            nc.vector.memset(epsb[:], 1e-10)

            apads = [
                apadp.tile([128, APW], bf16, tag=f"apad{i}", name=f"apad{i}")
                for i in range(2)
            ]
            for a_t in apads:
                nc.vector.memset(a_t[:, 0:PAD], 0.0)
                nc.vector.memset(a_t[:, PAD + N : APW], 0.0)
            lrs = [
                lrp.tile([128, N], bf16, tag=f"lr{i}", name=f"lr{i}") for i in range(2)
            ]
            psums = [
                psp.tile([128, 2048], f32, tag=f"ps{i}", name=f"ps{i}")
                for i in range(2)
            ]

            # ---- stencil lhsT matrices: Mv[p,l]: 1.0 at p==l+1, -0.25 at p==l, l+2
            #      NI[p,l]: -0.25 at p==l+1
            Mv = cp.tile([128, 126], bf16)
            NI = cp.tile([128, 126], bf16)
            idx = cp.tile([128, 126], i32)
            nc.gpsimd.iota(idx[:], pattern=[[-1, 126]], base=0, channel_multiplier=1)
            idxf = cp.tile([128, 126], f32)
            nc.gpsimd.tensor_copy(idxf[:], idx[:])
            vm1 = cp.tile([128, 126], f32)
            nc.vector.tensor_scalar(vm1[:], idxf[:], 1.0, None, Alu.subtract)  # p-l-1
            vab = cp.tile([128, 126], f32)
            vneg = cp.tile([128, 126], f32)
            nc.vector.tensor_scalar(vneg[:], vm1[:], -1.0, None, Alu.mult)
            nc.vector.tensor_max(vab[:], vm1[:], vneg[:])                      # |p-l-1|
            near = cp.tile([128, 126], f32)
            nc.vector.tensor_scalar(near[:], vab[:], 1.0, None, Alu.is_le)     # |.|<=1
            ctr = cp.tile([128, 126], f32)
            nc.vector.tensor_scalar(ctr[:], vab[:], 0.0, None, Alu.is_equal)   # ==0
            near4 = cp.tile([128, 126], f32)
            nc.vector.tensor_scalar(near4[:], near[:], 0.25, None, Alu.mult)
            ctr125 = cp.tile([128, 126], f32)
            nc.vector.tensor_scalar(ctr125[:], ctr[:], 1.25, None, Alu.mult)
            nc.vector.tensor_sub(Mv[:], ctr125[:], near4[:])
            nc.vector.tensor_scalar(NI[:], ctr[:], -0.25, None, Alu.mult)

            # ---- identity for the PE transpose of d
            iden_i = cp.tile([128, 128], i32)
            nc.gpsimd.iota(
                iden_i[:], pattern=[[-1, 128]], base=0, channel_multiplier=1
            )
            idnf = cp.tile([128, 128], f32)
            nc.gpsimd.tensor_copy(idnf[:], iden_i[:])
            idn0 = cp.tile([128, 128], f32)
            nc.vector.tensor_scalar(idn0[:], idnf[:], 0.0, None, Alu.is_equal)
            idn = cp.tile([128, 128], bf16)
            nc.vector.tensor_copy(idn[:], idn0[:])

            # ---- decay band constant: D[p,u] = exp(-0.1*|1088 + p - u|)
            decayb = cp.tile([128, BW], bf16)
            bidx = scrp.tile([128, BW], i32, tag="djunk", name="bidx")
            nc.gpsimd.iota(bidx[:], pattern=[[-1, BW]], base=1088, channel_multiplier=1)
            bidf = scrp.tile([128, BW], f32, tag="sabs", name="bidf")
            nc.gpsimd.tensor_copy(bidf[:], bidx[:])
            babs = scrp.tile([128, BW], f32, tag="djunk", name="babs")
            nc.scalar.activation(babs[:], bidf[:], Act.Abs)
            nc.scalar.activation(decayb[:], babs[:], Act.Exp, scale=-ALPHA)

            # ---- pass 1: d = row sums over all 1026 local rows (adj is relu'd
            # host-side). Split ACT Copy(accum_out) / DVE reduce halves.
            # Overlap rows get identical full-row sums; host masks dedupe.
            d_a = cp.tile([128, 16], f32)
            nc.vector.memset(d_a[:], 0.0)
            d_b = cp.tile([128, 16], f32)
            nc.vector.memset(d_b[:], 0.0)
            p1dst = scrp.tile([128, 4608], bf16, tag="ajunk", name="p1dst")
            for k, r0 in enumerate(R0S):
                t = iop.tile([128, N], bf16, tag="adj", name=f"p1t{k}")
                eng = nc.gpsimd if k % 2 else nc.sync
                eng.dma_start(t[:], adj_in[r0 : r0 + 128, :])
                nc.scalar.activation(
                    p1dst[:, 0:4608], t[:, 0:4608], Act.Copy,
                    accum_out=d_a[:, k : k + 1],
                )
                nc.vector.tensor_reduce(
                    d_b[:, k : k + 1], t[:, 4608:N], axis=X, op=Alu.add
                )
            d_tot = cp.tile([128, 16], f32)
            nc.vector.tensor_add(d_tot[:], d_a[:], d_b[:])

            # ---- AllGather of own d (local rows 1..1024 = global shard rows).
            # d is transposed to row-major via a PE matmul against the identity
            # (partition-strided 4-byte DMAs are ~8us each; this is 2 DMAs).
            d_totb = cp.tile([128, 16], bf16)
            nc.vector.tensor_copy(d_totb[:], d_tot[:])
            nc.tensor.matmul(
                psums[0][0:16, 0:128], d_totb[:], idn[:], start=True, stop=True
            )
            dT = cp.tile([16, 128], f32)
            nc.vector.tensor_copy(dT[:], psums[0][0:16, 0:128])
            dcore = dram.tile([1, SH], f32)
            nc.scalar.dma_start(
                dcore[0:1, 0:1008].rearrange("o (k p) -> (o k) p", k=8),
                dT[0:8, 1:127],
            )
            nc.scalar.dma_start(dcore[0:1, 1008:1024], dT[8:9, 111:127])
            dglob = dram.tile([NC, SH], f32)
            nc.gpsimd.collective_compute(
                "AllGather",
                Alu.bypass,
                replica_groups=[list(range(NC))],
                ins=[dcore.opt()],
                outs=[dglob.opt()],
            )

            # (overlaps the collective) local dinv for the stencil lhsT
            lnd = cp.tile([128, 16], f32)
            nc.scalar.activation(lnd[:, 0:9], d_tot[:, 0:9], Act.Ln, bias=epsb[:])
            dinv_sb = cp.tile([128, 16], f32)
            nc.scalar.activation(dinv_sb[:, 0:9], lnd[:, 0:9], Act.Exp, scale=-0.5)
            # d halves into the output accumulator for the host
            nc.vector.tensor_copy(acc[:, DA_COL : DA_COL + 9], d_a[:, 0:9])
            nc.vector.tensor_copy(acc[:, DB_COL : DB_COL + 9], d_b[:, 0:9])

            # ---- global column factors -> bf16 colfac tile [128, N]
            dg = cp.tile([128, 64], f32)
            nc.scalar.dma_start(
                dg[:],
                dglob[:].rearrange("a b -> (a b)").rearrange("(p t) -> p t", p=128),
            )
            lng = cp.tile([128, 64], f32)
            nc.scalar.activation(lng[:], dg[:], Act.Ln, bias=epsb[:])
            dgi = cp.tile([128, 64], f32)
            nc.scalar.activation(dgi[:], lng[:], Act.Exp, scale=-0.5)
            dgib = cp.tile([128, 64], bf16)
            nc.vector.tensor_copy(dgib[:], dgi[:])
            dinv1 = dram.tile([1, N], bf16)
            nc.scalar.dma_start(
                dinv1[0:1, :].rearrange("o (p t) -> (o p) t", p=128), dgib[:]
            )
            colfac = cp.tile([128, N], bf16)
            nc.sync.dma_start(
                colfac[:, 0 : N // 2], dinv1[0:1, 0 : N // 2].to_broadcast((128, N // 2))
            )
            nc.scalar.dma_start(
                colfac[:, N // 2 : N],
                dinv1[0:1, N // 2 : N].to_broadcast((128, N // 2)),
            )

            # ---- pass 2 (software-pipelined: tile k+1's A-build is emitted on
            # the in-order DVE queue BEFORE tile k's accumulation ops, so the
            # next tile's matmuls are never stuck behind DVE work that waits
            # on PE; psum edge zeroing runs on ACT, whose sabs read waits for
            # the same matmul group anyway)
            pid = nc.vector.partition_id()
            zcol = cp.tile([128, 1], f32)
            nc.vector.memset(zcol[:], 0.0)

            adj_ts = [None] * NT
            Mvks = [None] * NT
            NIks = [None] * NT

            def emit_dma(k):
                adj_ts[k] = iop.tile([128, N], bf16, tag="adj", name=f"adj{k}")
                eng = nc.gpsimd if k % 2 else nc.sync
                eng.dma_start(adj_ts[k][:], adj_in[R0S[k] : R0S[k] + 128, :])

            def emit_abuild(k):
                Apad = apads[k % 2]
                # A1c = adj * colfac (column factors only; row factor folded
                # into lhsT / host scaling)
                nc.vector.tensor_tensor(
                    Apad[:, CB : CB + N], adj_ts[k][:], colfac[:], Alu.mult
                )
                # lr = shift_left(A1c) + shift_right(A1c)
                nc.vector.tensor_tensor(
                    lrs[k % 2][:], Apad[:, CB - 1 : CB - 1 + N],
                    Apad[:, CB + 1 : CB + 1 + N], Alu.add
                )
                # row-scaled lhsT
                Mvks[k] = scrp.tile([128, 126], bf16, tag=f"mvk{k%2}", name=f"mvk{k}")
                NIks[k] = scrp.tile([128, 126], bf16, tag=f"nik{k%2}", name=f"nik{k}")
                nc.vector.tensor_scalar(
                    Mvks[k][:], Mv[:], dinv_sb[:, k : k + 1], None, Alu.mult
                )
                nc.vector.tensor_scalar(
                    NIks[k][:], NI[:], dinv_sb[:, k : k + 1], None, Alu.mult
                )

            def emit_accums(k):
                Apad = apads[k % 2]
                # DVE half of sum A1c^2 (row partials; host scales by dinv^2)
                sq = scrp.tile([128, N // 2], bf16, tag="djunk", name=f"sq{k}")
                nc.vector.scalar_tensor_tensor(
                    sq[:],
                    Apad[:, CB : CB + N // 2],
                    1.0,
                    Apad[:, CB : CB + N // 2],
                    Alu.bypass,
                    Alu.mult,
                    accum_out=acc[:, A2A_COL + k : A2A_COL + k + 1],
                )
                # band sum A1c*decay (row partials; host scales by dinv)
                bscr = scrp.tile([128, BW], bf16, tag="djunk", name=f"bscr{k}")
                nc.vector.scalar_tensor_tensor(
                    bscr[:],
                    Apad[:, bass.ds(pid * SH + (R0S[k] + 63), BW)],
                    1.0,
                    decayb[:],
                    Alu.bypass,
                    Alu.mult,
                    accum_out=acc[:, BD_COL + k : BD_COL + k + 1],
                )

            emit_dma(0)
            emit_dma(1)
            emit_abuild(0)
            for k, r0 in enumerate(R0S):
                Apad = apads[k % 2]
                lr = lrs[k % 2]
                # ACT half of sum A1c^2 first: it fills ACT's idle time while
                # PE runs this tile's matmuls (ACT's sabs reads wait on PE
                # groups regardless), and keeps it off the final-tile tail
                sqb = scrp.tile([128, N // 2], bf16, tag="ajunk", name=f"sqb{k}")
                nc.scalar.activation(
                    sqb[:],
                    Apad[:, CB + N // 2 : CB + N],
                    Act.Square,
                    accum_out=acc[:, A2B_COL + k : A2B_COL + k + 1],
                )
                if k == NT - 1:
                    # last tile: no next A-build to pipeline; run the DVE
                    # accums under the matmuls instead of after them
                    emit_accums(k)
                # stencil: t = A - 0.25*(up+down+left+right), 2 matmuls/chunk
                for q in range(4):
                    ps = psums[q % 2]
                    for cc in range(4):
                        c = 4 * q + cc
                        col = CB + 512 * c
                        out_ap = ps[0:126, 512 * cc : 512 * cc + 512]
                        nc.tensor.matmul(
                            out_ap, Mvks[k][:], Apad[:, col : col + 512],
                            start=True, stop=False,
                        )
                        nc.tensor.matmul(
                            out_ap, NIks[k][:], lr[:, 512 * c : 512 * c + 512],
                            start=False, stop=True,
                        )
                    if q == 0:
                        nc.scalar.activation(ps[0:126, 0:1], zcol[0:126, :], Act.Copy)
                    if q == 3:
                        nc.scalar.activation(
                            ps[0:126, 2047:2048], zcol[0:126, :], Act.Copy
                        )
                    sabs = scrp.tile([126, 2048], bf16, tag="sabs", name=f"sabs{k}_{q}")
                    nc.scalar.activation(
                        sabs[:], ps[0:126, :], Act.Abs,
                        accum_out=acc[0:126, SM_COL + 4 * k + q : SM_COL + 4 * k + q + 1],
                    )

                if k + 2 < NT:
                    emit_dma(k + 2)
                if k + 1 < NT:
                    emit_abuild(k + 1)
                if k < NT - 1:
                    emit_accums(k)

            acc2 = cp.tile([128, ACC_W], f32)
            nc.vector.tensor_copy(acc2[:], acc[:])
            nc.sync.dma_start(res_out[:], acc2[:])

    legalize_waits(nc)
    nc.finalize()
    drop_broken_range_clear(nc)
    return nc


def _masks():
    """Row-ownership masks resolving overlap-tile double counting (per core),
    plus global row index per (core, partition, tile)."""
    sm = np.zeros((NC, 128, 36), np.float64)
    rows = np.zeros((NC, 128, 9), np.float64)
    grow = np.zeros((NC, 128, 9), np.int64)
    for c in range(NC):
        claimed_r = set()
        claimed_s = set()
        for k, r0 in enumerate(R0S):
            for p in range(128):
                L = r0 + p
                g = SH * c - 1 + L
                grow[c, p, k] = min(max(g, 0), N - 1)
                if 1 <= L <= 1024 and L not in claimed_r:
                    claimed_r.add(L)
                    rows[c, p, k] = 1.0
            for p in range(126):
                L = r0 + 1 + p           # stencil out row (local)
                g = SH * c - 1 + L       # global row
                if 1 <= L <= 1024 and 1 <= g <= N - 2 and L not in claimed_s:
                    claimed_s.add(L)
                    sm[c, p, 4 * k : 4 * k + 4] = 1.0
    return sm, rows, grow


_SM_MASK, _ROW_MASK, _GROW = _masks()


def _analytic_decay_sq():
    k = np.arange(1, N, dtype=np.float64)
    return N + 2.0 * np.sum((N - k) * np.exp(-2.0 * ALPHA * k))


_NC_CACHE = None


def _prepare_in_maps(adj):
    import ml_dtypes

    in_maps = []
    for c in range(NC):
        sl = np.zeros((LR, N), ml_dtypes.bfloat16)
        lo = SH * c - 1
        src_lo = max(lo, 0)
        src_hi = min(lo + LR, N)
        sl[src_lo - lo : src_hi - lo, :] = np.maximum(adj[src_lo:src_hi], 0).astype(
            ml_dtypes.bfloat16
        )
        in_maps.append({"adj_sh": sl})
    return in_maps


def _reduce(results):
    # reconstruct per-global-row degree from the returned halves
    d_g = np.zeros(N, np.float64)
    parts = []
    for c in range(NC):
        o = results[c]["res"].astype(np.float64)
        parts.append(o)
        d = o[:, DA_COL : DA_COL + 9] + o[:, DB_COL : DB_COL + 9]
        np.add.at(d_g, _GROW[c], _ROW_MASK[c] * d)
    dinv_g = 1.0 / np.sqrt(d_g + 1e-10)

    s_sm = 0.0
    s_a2 = 0.0
    s_bd = 0.0
    for c in range(NC):
        o = parts[c]
        dv = dinv_g[_GROW[c]]                      # [128, 9]
        s_sm += float((o[:, SM_COL : SM_COL + 36] * _SM_MASK[c]).sum())
        a2 = o[:, A2A_COL : A2A_COL + 9] + o[:, A2B_COL : A2B_COL + 9]
        s_a2 += float((a2 * _ROW_MASK[c] * dv * dv).sum())
        s_bd += float((o[:, BD_COL : BD_COL + 9] * _ROW_MASK[c] * dv).sum())

    d2 = _analytic_decay_sq()
    loss = (s_a2 - 2.0 * s_bd + d2) + LAM * s_sm + GAMMA * s_a2
    return np.array(loss, dtype=np.float32)


def kernel(adj):
    global _NC_CACHE
    adj = np.ascontiguousarray(np.asarray(adj), dtype=np.float32)
    assert adj.shape == (N, N)

    if _NC_CACHE is None:
        _NC_CACHE = _build_nc()
    nc = _NC_CACHE

    res = run_bass_kernel_spmd(nc, _prepare_in_maps(adj), core_ids=list(range(NC)))
    return _reduce(res.results)


# revision 23
# speedup vs baseline: 1.0069x; 1.0069x over previous
"""Trainium2 Bass kernel for nn_CombinedGraphLoss (graph-loss over 8192x8192 adj).

loss = sum((A - decay)^2) + 0.1*sum|A - mean4(A)| + 0.001*sum(A^2)
with A = D^-1/2 relu(adj) D^-1/2, decay = exp(-0.1|i-j|).

Strategy (8 cores, row-sharded, full inputs per core):
  - each core gets relu(its 1024-row shard + 1 halo row each side) pre-converted
    to bf16 on the host; both passes stream the same 9 overlapping 128-row
    tiles (stride 126) from HBM.
  - pass1: row sums d, split ACT Copy(accum_out) / DVE reduce halves.
  - AllGather d -> global column factors colfac = exp(-0.5*ln(d+eps)) (bf16).
  - pass2 computes COLUMN-scaled A1c = adj*colfac only (one DVE op); the row
    factor dinv_i is folded into the stencil lhsT matrices on device and into
    the per-row partial sums on the host (f64):
      * stencil t = A - 0.25*(up+down+left+right) via 2 matmuls per 512-col
        chunk: tridiagonal Mv*dinv_i for center+vertical, diagonal
        NI*dinv_i = -0.25*dinv_i*I applied to lr = shift_left(A1c)+
        shift_right(A1c) (DVE); |t| row-sums via ACT Abs(accum_out) from PSUM.
      * sum A1c^2 row partials (DVE STT half / ACT Square half) -> host*dinv^2
      * band sum A1c*decay row partials (decay==0 in fp32 outside |i-j|>1039;
        2304-wide window, dynamic AP at pid-dependent offset) -> host*dinv
      * row sums d returned -> host computes dinv in f64.
  - decay terms decomposed: sum(A-decay)^2 = sumA^2 - 2*sum(A*decay) + sum(decay^2);
    sum decay^2 is analytic on host.
  - host applies row-ownership masks (overlap tiles) and reduces in float64.

All heavy elementwise work runs on DVE/ACT only -- GpSimd (Pool) executes no
big tensor ops (its software loops are ~25x slower and starve DVE through the
shared SBUF ports, which was the old kernel's bottleneck).

The wait-legalization passes below work around this toolchain's walrus, which
rejects instructions carrying more than one semaphore wait and miscompiles
EVENT_SEMAPHORE_RANGE_CLEAR.
"""

import numpy as np

import concourse.bass as bass
import concourse.mybir as mybir
from concourse import tile
from concourse.bass_utils import run_bass_kernel_spmd

from collections import defaultdict


def _facts_union(a, b):
    # facts: dict sem_id -> max value known reached
    for s, v in b.items():
        if a.get(s, -1) < v:
            a[s] = v
    return a


def strip_redundant_waits(nc, verbose=False):
    insts = []
    for bb in nc.m.functions[0].blocks:
        insts.extend(bb.instructions)

    # classify sems: updated by exactly one engine-proc (in-order) or not
    sem_updaters = defaultdict(set)
    for ins in insts:
        si = ins.sync_info
        if si is None:
            continue
        eng = getattr(ins, "engine", None)
        is_dma = type(ins).__name__ == "InstDMACopy"
        proc = ("dma", getattr(ins, "queue", "")) if is_dma else ("eng", str(eng))
        for u in si.on_update:
            sem_updaters[u.id].add(proc)
    inorder_sem = {
        s: next(iter(p))
        for s, p in sem_updaters.items()
        if len(p) == 1 and next(iter(p))[0] == "eng"
    }

    # walk in emission order, tracking per-proc facts and per-sem crossing facts
    proc_facts = defaultdict(dict)          # proc -> facts
    sem_cum = defaultdict(int)              # sem -> cumulative value
    sem_cross = defaultdict(list)           # sem -> [(cum_after, facts)]
    n_stripped = 0
    max_left = 0

    for ins in insts:
        si = ins.sync_info
        if si is None:
            continue
        eng = getattr(ins, "engine", None)
        is_dma = type(ins).__name__ == "InstDMACopy"
        proc = ("dma", getattr(ins, "queue", "")) if is_dma else ("eng", str(eng))
        in_order = not is_dma

        def wait_facts(w):
            # facts implied by "sem w.id >= w.value" holding
            f = {w.id: w.wait_value}
            if w.id in inorder_sem:
                for cum, facts in sem_cross[w.id]:
                    if cum >= w.wait_value:
                        _facts_union(f, facts)
                        break
            return f

        waits = list(si.on_wait)
        if len(waits) > 1:
            base = dict(proc_facts[proc]) if in_order else {}
            # engine-sem waits are always kept; other waits are dropped when
            # implied by program order + the kept engine-sem waits
            for w in waits:
                if w.id in inorder_sem:
                    _facts_union(base, wait_facts(w))
            keep = []
            drop = []
            for w in waits:
                if w.id not in inorder_sem and base.get(w.id, -1) >= w.wait_value:
                    drop.append(w)
                else:
                    keep.append(w)
            if drop:
                n_stripped += len(drop)
                from concourse import mybir

                ins.sync_info = mybir.SyncInfo(
                    on_wait=keep, on_update=list(si.on_update)
                )
                si = ins.sync_info
            waits = keep
        max_left = max(max_left, len(waits))

        # facts after this instruction completes
        myf = dict(proc_facts[proc]) if in_order else {}
        for w in waits:
            _facts_union(myf, wait_facts(w))
        for u in si.on_update:
            sem_cum[u.id] += u.update_value
            f = dict(myf)
            f[u.id] = sem_cum[u.id]
            sem_cross[u.id].append((sem_cum[u.id], f))
            if in_order:
                # own-sem value is part of this proc's program-order knowledge
                myf[u.id] = sem_cum[u.id]
        if in_order:
            proc_facts[proc] = myf

    if verbose:
        print(f"waitstrip: removed {n_stripped} waits, max remaining {max_left}")
    return n_stripped, max_left


def split_multi_waits(nc, verbose=False):
    """Rewrite instructions carrying >1 sync wait into a chain of same-engine
    NOPs each carrying one wait (in-order engine queues make this equivalent).
    Must run after strip_redundant_waits. DMACopy must already be single-wait.
    """
    from concourse import mybir

    n_split = 0
    for bb_w in nc.m.functions[0].blocks:
        il = bb_w.instructions
        i = 0
        while i < len(il):
            ins = il[i]
            si = ins.sync_info
            if si is not None and len(si.on_wait) > 1:
                # DMACopy here is SWDGE (engine=Pool): descriptor generation
                # runs in the Pool instruction stream, so a preceding Pool nop
                # legally gates it just like any compute instruction.
                waits = list(si.on_wait)
                extra, keep = waits[:-1], waits[-1:]
                for w in extra:
                    r = nc.engines[ins.engine].nop()
                    # pull the freshly appended nop out of whichever bb got it
                    nop_ins = r.ins
                    removed = False
                    for bb2 in nc.m.functions[0].blocks:
                        il2 = bb2.instructions
                        if il2 and il2[-1] is nop_ins:
                            il2.pop()
                            removed = True
                            break
                    assert removed, "could not locate appended nop"
                    nop_ins.sync_info = mybir.SyncInfo(on_wait=[w], on_update=[])
                    il.insert(i, nop_ins)
                    i += 1
                    n_split += 1
                ins.sync_info = mybir.SyncInfo(
                    on_wait=keep, on_update=list(si.on_update)
                )
            i += 1
    if verbose:
        print(f"waitstrip: split {n_split} waits onto nops")
    return n_split


def drop_broken_range_clear(nc, verbose=False):
    """This walrus snapshot miscompiles EVENT_SEMAPHORE_RANGE_CLEAR ("ISA
    wrong length"). It only matters for re-executing an already-loaded NEFF
    with dirty semaphores; drop it (verified empirically with back-to-back
    executions)."""
    n = 0
    for bb_w in nc.m.functions[0].blocks:
        il = bb_w.instructions
        for i in range(len(il) - 1, -1, -1):
            ins = il[i]
            if type(ins).__name__ == "InstISA" and getattr(ins, "isa_opcode", 0) == 176:
                del il[i]
                n += 1
    if verbose:
        print(f"waitstrip: dropped {n} EVENT_SEMAPHORE_RANGE_CLEAR")


def legalize_waits(nc, verbose=False):
    drop_broken_range_clear(nc, verbose=verbose)
    strip_redundant_waits(nc, verbose=verbose)
    split_multi_waits(nc, verbose=verbose)
    bad = []
    for bb_w in nc.m.functions[0].blocks:
        for ins in bb_w.instructions:
            si = ins.sync_info
            if si is not None and len(si.on_wait) > 1:
                bad.append(ins.name)
    assert not bad, f"instructions still multi-wait: {bad}"


N = 8192
NC = 8
SH = N // NC          # 1024 rows per core
LR = SH + 2           # local rows incl halos = 1026
ALPHA = 0.1
LAM = 0.1
GAMMA = 0.001

BW = 2304             # band width (covers |i-j| <= 1088 for every tile row)
PAD = 1152            # zero padding on each side of A1c_pad
APW = N + 2 * PAD     # 10496
CB = PAD              # first real column inside A1c_pad
R0S = [126 * k for k in range(8)] + [LR - 128]   # tile starts (local rows)
NT = len(R0S)

f32 = mybir.dt.float32
bf16 = mybir.dt.bfloat16
i32 = mybir.dt.int32
Alu = mybir.AluOpType
Act = mybir.ActivationFunctionType
X = mybir.AxisListType.X

# accumulator column layout in the [128, 96] f32 output
SM_COL = 0     # 36 cols: tile k quarter q -> 4k+q, rows 0..125 (no host scale)
A2A_COL = 36   # 9 cols: DVE half of sum A1c^2 (host * dinv^2)
A2B_COL = 45   # 9 cols: ACT half of sum A1c^2 (host * dinv^2)
BD_COL = 54    # 9 cols: band sum A1c*decay (host * dinv)
DA_COL = 63    # 9 cols: ACT half of row sums d
DB_COL = 72    # 9 cols: DVE half of row sums d
ACC_W = 96


def _build_nc():
    nc = bass.Bass(num_devices=NC)
    adj_in = nc.dram_tensor("adj_sh", [LR, N], bf16, kind="ExternalInput")
    res_out = nc.dram_tensor("res", [128, ACC_W], f32, kind="ExternalOutput")

    with tile.TileContext(nc) as tc:
        with (
            tc.tile_pool(name="const", bufs=1) as cp,
            tc.tile_pool(name="dram", bufs=1, space="DRAM") as dram,
            tc.tile_pool(name="io", bufs=3) as iop,
            tc.tile_pool(name="apad", bufs=1) as apadp,
            tc.tile_pool(name="lr", bufs=1) as lrp,
            tc.tile_pool(name="scr", bufs=1) as scrp,
            tc.tile_pool(name="ps", bufs=1, space="PSUM") as psp,
        ):
            acc = cp.tile([128, ACC_W], f32)
            nc.vector.memset(acc[:], 0.0)
            epsb = cp.tile([128, 1], f32)
            nc.vector.memset(epsb[:], 1e-10)

            apads = [
                apadp.tile([128, APW], bf16, tag=f"apad{i}", name=f"apad{i}")
                for i in range(2)
            ]
            for a_t in apads:
                nc.vector.memset(a_t[:, 0:PAD], 0.0)
                nc.vector.memset(a_t[:, PAD + N : APW], 0.0)
            lrs = [
                lrp.tile([128, N], bf16, tag=f"lr{i}", name=f"lr{i}") for i in range(2)
            ]
            psums = [
                psp.tile([128, 2048], f32, tag=f"ps{i}", name=f"ps{i}")
                for i in range(2)
            ]

            # ---- stencil lhsT matrices: Mv[p,l]: 1.0 at p==l+1, -0.25 at p==l, l+2
            #      NI[p,l]: -0.25 at p==l+1
            Mv = cp.tile([128, 126], bf16)
            NI = cp.tile([128, 126], bf16)
            idx = cp.tile([128, 126], i32)
            nc.gpsimd.iota(idx[:], pattern=[[-1, 126]], base=0, channel_multiplier=1)
            idxf = cp.tile([128, 126], f32)
            nc.gpsimd.tensor_copy(idxf[:], idx[:])
            vm1 = cp.tile([128, 126], f32)
            nc.vector.tensor_scalar(vm1[:], idxf[:], 1.0, None, Alu.subtract)  # p-l-1
            vab = cp.tile([128, 126], f32)
            vneg = cp.tile([128, 126], f32)
            nc.vector.tensor_scalar(vneg[:], vm1[:], -1.0, None, Alu.mult)
            nc.vector.tensor_max(vab[:], vm1[:], vneg[:])                      # |p-l-1|
            near = cp.tile([128, 126], f32)
            nc.vector.tensor_scalar(near[:], vab[:], 1.0, None, Alu.is_le)     # |.|<=1
            ctr = cp.tile([128, 126], f32)
            nc.vector.tensor_scalar(ctr[:], vab[:], 0.0, None, Alu.is_equal)   # ==0
            near4 = cp.tile([128, 126], f32)
            nc.vector.tensor_scalar(near4[:], near[:], 0.25, None, Alu.mult)
            ctr125 = cp.tile([128, 126], f32)
            nc.vector.tensor_scalar(ctr125[:], ctr[:], 1.25, None, Alu.mult)
            nc.vector.tensor_sub(Mv[:], ctr125[:], near4[:])
            nc.vector.tensor_scalar(NI[:], ctr[:], -0.25, None, Alu.mult)

            # ---- identity for the PE transpose of d
            iden_i = cp.tile([128, 128], i32)
            nc.gpsimd.iota(
                iden_i[:], pattern=[[-1, 128]], base=0, channel_multiplier=1
            )
            idnf = cp.tile([128, 128], f32)
            nc.gpsimd.tensor_copy(idnf[:], iden_i[:])
            idn0 = cp.tile([128, 128], f32)
            nc.vector.tensor_scalar(idn0[:], idnf[:], 0.0, None, Alu.is_equal)
            idn = cp.tile([128, 128], bf16)
            nc.vector.tensor_copy(idn[:], idn0[:])

            # ---- decay band constant: D[p,u] = exp(-0.1*|1088 + p - u|)
            decayb = cp.tile([128, BW], bf16)
            bidx = scrp.tile([128, BW], i32, tag="djunk", name="bidx")
            nc.gpsimd.iota(bidx[:], pattern=[[-1, BW]], base=1088, channel_multiplier=1)
            bidf = scrp.tile([128, BW], f32, tag="sabs", name="bidf")
            nc.gpsimd.tensor_copy(bidf[:], bidx[:])
            babs = scrp.tile([128, BW], f32, tag="djunk", name="babs")
            nc.scalar.activation(babs[:], bidf[:], Act.Abs)
            nc.scalar.activation(decayb[:], babs[:], Act.Exp, scale=-ALPHA)

            # ---- pass 1: d = row sums over all 1026 local rows (adj is relu'd
            # host-side). Split ACT Copy(accum_out) / DVE reduce halves.
            # Overlap rows get identical full-row sums; host masks dedupe.
            d_a = cp.tile([128, 16], f32)
            nc.vector.memset(d_a[:], 0.0)
            d_b = cp.tile([128, 16], f32)
            nc.vector.memset(d_b[:], 0.0)
            p1dst = scrp.tile([128, 4608], bf16, tag="ajunk", name="p1dst")
            for k, r0 in enumerate(R0S):
                t = iop.tile([128, N], bf16, tag="adj", name=f"p1t{k}")
                eng = nc.gpsimd if k % 2 else nc.sync
                eng.dma_start(t[:], adj_in[r0 : r0 + 128, :])
                nc.scalar.activation(
                    p1dst[:, 0:4608], t[:, 0:4608], Act.Copy,
                    accum_out=d_a[:, k : k + 1],
                )
                nc.vector.tensor_reduce(
                    d_b[:, k : k + 1], t[:, 4608:N], axis=X, op=Alu.add
                )
            d_tot = cp.tile([128, 16], f32)
            nc.vector.tensor_add(d_tot[:], d_a[:], d_b[:])

            # ---- AllGather of own d (local rows 1..1024 = global shard rows).
            # d is transposed to row-major via a PE matmul against the identity
            # (partition-strided 4-byte DMAs are ~8us each; this is 2 DMAs).
            d_totb = cp.tile([128, 16], bf16)
            nc.vector.tensor_copy(d_totb[:], d_tot[:])
            nc.tensor.matmul(
                psums[0][0:16, 0:128], d_totb[:], idn[:], start=True, stop=True
            )
            dT = cp.tile([16, 128], f32)
            nc.vector.tensor_copy(dT[:], psums[0][0:16, 0:128])
            dcore = dram.tile([1, SH], f32)
            nc.scalar.dma_start(
                dcore[0:1, 0:1008].rearrange("o (k p) -> (o k) p", k=8),
                dT[0:8, 1:127],
            )
            nc.scalar.dma_start(dcore[0:1, 1008:1024], dT[8:9, 111:127])
            dglob = dram.tile([NC, SH], f32)
            nc.gpsimd.collective_compute(
                "AllGather",
                Alu.bypass,
                replica_groups=[list(range(NC))],
                ins=[dcore.opt()],
                outs=[dglob.opt()],
            )

            # (overlaps the collective) local dinv for the stencil lhsT
            lnd = cp.tile([128, 16], f32)
            nc.scalar.activation(lnd[:, 0:9], d_tot[:, 0:9], Act.Ln, bias=epsb[:])
            dinv_sb = cp.tile([128, 16], f32)
            nc.scalar.activation(dinv_sb[:, 0:9], lnd[:, 0:9], Act.Exp, scale=-0.5)
            # d halves into the output accumulator for the host
            nc.vector.tensor_copy(acc[:, DA_COL : DA_COL + 9], d_a[:, 0:9])
            nc.vector.tensor_copy(acc[:, DB_COL : DB_COL + 9], d_b[:, 0:9])

            # ---- global column factors -> bf16 colfac tile [128, N]
            dg = cp.tile([128, 64], f32)
            nc.scalar.dma_start(
                dg[:],
                dglob[:].rearrange("a b -> (a b)").rearrange("(p t) -> p t", p=128),
            )
            lng = cp.tile([128, 64], f32)
            nc.scalar.activation(lng[:], dg[:], Act.Ln, bias=epsb[:])
            dgi = cp.tile([128, 64], f32)
            nc.scalar.activation(dgi[:], lng[:], Act.Exp, scale=-0.5)
            dgib = cp.tile([128, 64], bf16)
            nc.vector.tensor_copy(dgib[:], dgi[:])
            dinv1 = dram.tile([1, N], bf16)
            nc.scalar.dma_start(
                dinv1[0:1, :].rearrange("o (p t) -> (o p) t", p=128), dgib[:]
            )
            colfac = cp.tile([128, N], bf16)
            nc.sync.dma_start(
                colfac[:, 0 : N // 2], dinv1[0:1, 0 : N // 2].to_broadcast((128, N // 2))
            )
            nc.scalar.dma_start(
                colfac[:, N // 2 : N],
                dinv1[0:1, N // 2 : N].to_broadcast((128, N // 2)),
            )

            # ---- pass 2 (software-pipelined: tile k+1's A-build is emitted on
            # the in-order DVE queue BEFORE tile k's accumulation ops, so the
            # next tile's matmuls are never stuck behind DVE work that waits
            # on PE; psum edge zeroing runs on ACT, whose sabs read waits for
            # the same matmul group anyway)
            pid = nc.vector.partition_id()
            zcol = cp.tile([128, 1], f32)
            nc.vector.memset(zcol[:], 0.0)

            adj_ts = [None] * NT
            Mvks = [None] * NT
            NIks = [None] * NT

            def emit_dma(k):
                adj_ts[k] = iop.tile([128, N], bf16, tag="adj", name=f"adj{k}")
                eng = nc.gpsimd if k % 2 else nc.sync
                eng.dma_start(adj_ts[k][:], adj_in[R0S[k] : R0S[k] + 128, :])

            def emit_abuild(k):
                Apad = apads[k % 2]
                # A1c = adj * colfac (column factors only; row factor folded
                # into lhsT / host scaling)
                nc.vector.tensor_tensor(
                    Apad[:, CB : CB + N], adj_ts[k][:], colfac[:], Alu.mult
                )
                # lr = shift_left(A1c) + shift_right(A1c)
                nc.vector.tensor_tensor(
                    lrs[k % 2][:], Apad[:, CB - 1 : CB - 1 + N],
                    Apad[:, CB + 1 : CB + 1 + N], Alu.add
                )
                # row-scaled lhsT
                Mvks[k] = scrp.tile([128, 126], bf16, tag=f"mvk{k%2}", name=f"mvk{k}")
                NIks[k] = scrp.tile([128, 126], bf16, tag=f"nik{k%2}", name=f"nik{k}")
                nc.vector.tensor_scalar(
                    Mvks[k][:], Mv[:], dinv_sb[:, k : k + 1], None, Alu.mult
                )
                nc.vector.tensor_scalar(
                    NIks[k][:], NI[:], dinv_sb[:, k : k + 1], None, Alu.mult
                )

            def emit_accums(k):
                Apad = apads[k % 2]
                # DVE half of sum A1c^2 (row partials; host scales by dinv^2)
                sq = scrp.tile([128, 2944], bf16, tag="djunk", name=f"sq{k}")
                nc.vector.scalar_tensor_tensor(
                    sq[:],
                    Apad[:, CB : CB + 2944],
                    1.0,
                    Apad[:, CB : CB + 2944],
                    Alu.bypass,
                    Alu.mult,
                    accum_out=acc[:, A2A_COL + k : A2A_COL + k + 1],
                )
                # band sum A1c*decay (row partials; host scales by dinv)
                bscr = scrp.tile([128, BW], bf16, tag="djunk", name=f"bscr{k}")
                nc.vector.scalar_tensor_tensor(
                    bscr[:],
                    Apad[:, bass.ds(pid * SH + (R0S[k] + 63), BW)],
                    1.0,
                    decayb[:],
                    Alu.bypass,
                    Alu.mult,
                    accum_out=acc[:, BD_COL + k : BD_COL + k + 1],
                )

            emit_dma(0)
            emit_dma(1)
            emit_abuild(0)
            for k, r0 in enumerate(R0S):
                Apad = apads[k % 2]
                lr = lrs[k % 2]
                # ACT half of sum A1c^2 first: it fills ACT's idle time while
                # PE runs this tile's matmuls (ACT's sabs reads wait on PE
                # groups regardless), and keeps it off the final-tile tail
                sqb = scrp.tile([128, 5248], bf16, tag="ajunk", name=f"sqb{k}")
                nc.scalar.activation(
                    sqb[:],
                    Apad[:, CB + 2944 : CB + N],
                    Act.Square,
                    accum_out=acc[:, A2B_COL + k : A2B_COL + k + 1],
                )
                if k == NT - 1:
                    # last tile: no next A-build to pipeline; run the DVE
                    # accums under the matmuls instead of after them
                    emit_accums(k)
                # stencil: t = A - 0.25*(up+down+left+right), 2 matmuls/chunk
                for q in range(4):
                    ps = psums[q % 2]
                    for cc in range(4):
                        c = 4 * q + cc
                        col = CB + 512 * c
                        out_ap = ps[0:126, 512 * cc : 512 * cc + 512]
                        nc.tensor.matmul(
                            out_ap, Mvks[k][:], Apad[:, col : col + 512],
                            start=True, stop=False,
                        )
                        nc.tensor.matmul(
                            out_ap, NIks[k][:], lr[:, 512 * c : 512 * c + 512],
                            start=False, stop=True,
                        )
                    if q == 0:
                        nc.scalar.activation(ps[0:126, 0:1], zcol[0:126, :], Act.Copy)
                    if q == 3:
                        nc.scalar.activation(
                            ps[0:126, 2047:2048], zcol[0:126, :], Act.Copy
                        )
                    sabs = scrp.tile([126, 2048], bf16, tag="sabs", name=f"sabs{k}_{q}")
                    nc.scalar.activation(
                        sabs[:], ps[0:126, :], Act.Abs,
                        accum_out=acc[0:126, SM_COL + 4 * k + q : SM_COL + 4 * k + q + 1],
                    )

                if k + 2 < NT:
                    emit_dma(k + 2)
                if k + 1 < NT:
                    emit_abuild(k + 1)
                if k < NT - 1:
                    emit_accums(k)

            acc2 = cp.tile([128, ACC_W], f32)
            nc.vector.tensor_copy(acc2[:], acc[:])
            nc.sync.dma_start(res_out[:], acc2[:])

    legalize_waits(nc)
    nc.finalize()
    drop_broken_range_clear(nc)
    return nc


def _masks():
    """Row-ownership masks resolving overlap-tile double counting (per core),
    plus global row index per (core, partition, tile)."""
    sm = np.zeros((NC, 128, 36), np.float64)
    rows = np.zeros((NC, 128, 9), np.float64)
    grow = np.zeros((NC, 128, 9), np.int64)
    for c in range(NC):
        claimed_r = set()
        claimed_s = set()
        for k, r0 in enumerate(R0S):
            for p in range(128):
                L = r0 + p
                g = SH * c - 1 + L
                grow[c, p, k] = min(max(g, 0), N - 1)
                if 1 <= L <= 1024 and L not in claimed_r:
                    claimed_r.add(L)
                    rows[c, p, k] = 1.0
            for p in range(126):
                L = r0 + 1 + p           # stencil out row (local)
                g = SH * c - 1 + L       # global row
                if 1 <= L <= 1024 and 1 <= g <= N - 2 and L not in claimed_s:
                    claimed_s.add(L)
                    sm[c, p, 4 * k : 4 * k + 4] = 1.0
    return sm, rows, grow


_SM_MASK, _ROW_MASK, _GROW = _masks()


def _analytic_decay_sq():
    k = np.arange(1, N, dtype=np.float64)
    return N + 2.0 * np.sum((N - k) * np.exp(-2.0 * ALPHA * k))


_NC_CACHE = None


def _prepare_in_maps(adj):
    import ml_dtypes

    in_maps = []
    for c in range(NC):
        sl = np.zeros((LR, N), ml_dtypes.bfloat16)
        lo = SH * c - 1
        src_lo = max(lo, 0)
        src_hi = min(lo + LR, N)
        sl[src_lo - lo : src_hi - lo, :] = np.maximum(adj[src_lo:src_hi], 0).astype(
            ml_dtypes.bfloat16
        )
        in_maps.append({"adj_sh": sl})
    return in_maps


def _reduce(results):
    # reconstruct per-global-row degree from the returned halves
    d_g = np.zeros(N, np.float64)
    parts = []
    for c in range(NC):
        o = results[c]["res"].astype(np.float64)
        parts.append(o)
        d = o[:, DA_COL : DA_COL + 9] + o[:, DB_COL : DB_COL + 9]
        np.add.at(d_g, _GROW[c], _ROW_MASK[c] * d)
    dinv_g = 1.0 / np.sqrt(d_g + 1e-10)

    s_sm = 0.0
    s_a2 = 0.0
    s_bd = 0.0
    for c in range(NC):
        o = parts[c]
        dv = dinv_g[_GROW[c]]                      # [128, 9]
        s_sm += float((o[:, SM_COL : SM_COL + 36] * _SM_MASK[c]).sum())
        a2 = o[:, A2A_COL : A2A_COL + 9] + o[:, A2B_COL : A2B_COL + 9]
        s_a2 += float((a2 * _ROW_MASK[c] * dv * dv).sum())
        s_bd += float((o[:, BD_COL : BD_COL + 9] * _ROW_MASK[c] * dv).sum())

    d2 = _analytic_decay_sq()
    loss = (s_a2 - 2.0 * s_bd + d2) + LAM * s_sm + GAMMA * s_a2
    return np.array(loss, dtype=np.float32)


def kernel(adj):
    global _NC_CACHE
    adj = np.ascontiguousarray(np.asarray(adj), dtype=np.float32)
    assert adj.shape == (N, N)

    if _NC_CACHE is None:
        _NC_CACHE = _build_nc()
    nc = _NC_CACHE

    res = run_bass_kernel_spmd(nc, _prepare_in_maps(adj), core_ids=list(range(NC)))
    return _reduce(res.results)


# revision 24
# speedup vs baseline: 1.0289x; 1.0218x over previous
"""Trainium2 Bass kernel for nn_CombinedGraphLoss (graph-loss over 8192x8192 adj).

loss = sum((A - decay)^2) + 0.1*sum|A - mean4(A)| + 0.001*sum(A^2)
with A = D^-1/2 relu(adj) D^-1/2, decay = exp(-0.1|i-j|).

Strategy (8 cores, row-sharded, full inputs per core):
  - each core gets relu(its 1024-row shard + 1 halo row each side) pre-converted
    to bf16 on the host; both passes stream the same 9 overlapping 128-row
    tiles (stride 126) from HBM.
  - pass1: row sums d, split ACT Copy(accum_out) / DVE reduce halves.
  - AllGather d -> global column factors colfac = exp(-0.5*ln(d+eps)) (bf16).
  - pass2 computes COLUMN-scaled A1c = adj*colfac only (one DVE op); the row
    factor dinv_i is folded into the stencil lhsT matrices on device and into
    the per-row partial sums on the host (f64):
      * stencil t = A - 0.25*(up+down+left+right) via 2 matmuls per 512-col
        chunk: tridiagonal Mv*dinv_i for center+vertical, diagonal
        NI*dinv_i = -0.25*dinv_i*I applied to lr = shift_left(A1c)+
        shift_right(A1c) (DVE); |t| row-sums via ACT Abs(accum_out) from PSUM.
      * sum A1c^2 row partials (DVE STT half / ACT Square half) -> host*dinv^2
      * band sum A1c*decay row partials (decay==0 in fp32 outside |i-j|>1039;
        2304-wide window, dynamic AP at pid-dependent offset) -> host*dinv
      * row sums d returned -> host computes dinv in f64.
  - decay terms decomposed: sum(A-decay)^2 = sumA^2 - 2*sum(A*decay) + sum(decay^2);
    sum decay^2 is analytic on host.
  - host applies row-ownership masks (overlap tiles) and reduces in float64.

All heavy elementwise work runs on DVE/ACT only -- GpSimd (Pool) executes no
big tensor ops (its software loops are ~25x slower and starve DVE through the
shared SBUF ports, which was the old kernel's bottleneck).

The wait-legalization passes below work around this toolchain's walrus, which
rejects instructions carrying more than one semaphore wait and miscompiles
EVENT_SEMAPHORE_RANGE_CLEAR.
"""

import numpy as np

import concourse.bass as bass
import concourse.mybir as mybir
from concourse import tile
from concourse.bass_utils import run_bass_kernel_spmd

from collections import defaultdict


def _facts_union(a, b):
    # facts: dict sem_id -> max value known reached
    for s, v in b.items():
        if a.get(s, -1) < v:
            a[s] = v
    return a


def strip_redundant_waits(nc, verbose=False):
    insts = []
    for bb in nc.m.functions[0].blocks:
        insts.extend(bb.instructions)

    # classify sems: updated by exactly one engine-proc (in-order) or not
    sem_updaters = defaultdict(set)
    for ins in insts:
        si = ins.sync_info
        if si is None:
            continue
        eng = getattr(ins, "engine", None)
        is_dma = type(ins).__name__ == "InstDMACopy"
        proc = ("dma", getattr(ins, "queue", "")) if is_dma else ("eng", str(eng))
        for u in si.on_update:
            sem_updaters[u.id].add(proc)
    inorder_sem = {
        s: next(iter(p))
        for s, p in sem_updaters.items()
        if len(p) == 1 and next(iter(p))[0] == "eng"
    }

    # walk in emission order, tracking per-proc facts and per-sem crossing facts
    proc_facts = defaultdict(dict)          # proc -> facts
    sem_cum = defaultdict(int)              # sem -> cumulative value
    sem_cross = defaultdict(list)           # sem -> [(cum_after, facts)]
    n_stripped = 0
    max_left = 0

    for ins in insts:
        si = ins.sync_info
        if si is None:
            continue
        eng = getattr(ins, "engine", None)
        is_dma = type(ins).__name__ == "InstDMACopy"
        proc = ("dma", getattr(ins, "queue", "")) if is_dma else ("eng", str(eng))
        in_order = not is_dma

        def wait_facts(w):
            # facts implied by "sem w.id >= w.value" holding
            f = {w.id: w.wait_value}
            if w.id in inorder_sem:
                for cum, facts in sem_cross[w.id]:
                    if cum >= w.wait_value:
                        _facts_union(f, facts)
                        break
            return f

        waits = list(si.on_wait)
        if len(waits) > 1:
            base = dict(proc_facts[proc]) if in_order else {}
            # engine-sem waits are always kept; other waits are dropped when
            # implied by program order + the kept engine-sem waits
            for w in waits:
                if w.id in inorder_sem:
                    _facts_union(base, wait_facts(w))
            keep = []
            drop = []
            for w in waits:
                if w.id not in inorder_sem and base.get(w.id, -1) >= w.wait_value:
                    drop.append(w)
                else:
                    keep.append(w)
            if drop:
                n_stripped += len(drop)
                from concourse import mybir

                ins.sync_info = mybir.SyncInfo(
                    on_wait=keep, on_update=list(si.on_update)
                )
                si = ins.sync_info
            waits = keep
        max_left = max(max_left, len(waits))

        # facts after this instruction completes
        myf = dict(proc_facts[proc]) if in_order else {}
        for w in waits:
            _facts_union(myf, wait_facts(w))
        for u in si.on_update:
            sem_cum[u.id] += u.update_value
            f = dict(myf)
            f[u.id] = sem_cum[u.id]
            sem_cross[u.id].append((sem_cum[u.id], f))
            if in_order:
                # own-sem value is part of this proc's program-order knowledge
                myf[u.id] = sem_cum[u.id]
        if in_order:
            proc_facts[proc] = myf

    if verbose:
        print(f"waitstrip: removed {n_stripped} waits, max remaining {max_left}")
    return n_stripped, max_left


def split_multi_waits(nc, verbose=False):
    """Rewrite instructions carrying >1 sync wait into a chain of same-engine
    NOPs each carrying one wait (in-order engine queues make this equivalent).
    Must run after strip_redundant_waits. DMACopy must already be single-wait.
    """
    from concourse import mybir

    n_split = 0
    for bb_w in nc.m.functions[0].blocks:
        il = bb_w.instructions
        i = 0
        while i < len(il):
            ins = il[i]
            si = ins.sync_info
            if si is not None and len(si.on_wait) > 1:
                # DMACopy here is SWDGE (engine=Pool): descriptor generation
                # runs in the Pool instruction stream, so a preceding Pool nop
                # legally gates it just like any compute instruction.
                waits = list(si.on_wait)
                extra, keep = waits[:-1], waits[-1:]
                for w in extra:
                    r = nc.engines[ins.engine].nop()
                    # pull the freshly appended nop out of whichever bb got it
                    nop_ins = r.ins
                    removed = False
                    for bb2 in nc.m.functions[0].blocks:
                        il2 = bb2.instructions
                        if il2 and il2[-1] is nop_ins:
                            il2.pop()
                            removed = True
                            break
                    assert removed, "could not locate appended nop"
                    nop_ins.sync_info = mybir.SyncInfo(on_wait=[w], on_update=[])
                    il.insert(i, nop_ins)
                    i += 1
                    n_split += 1
                ins.sync_info = mybir.SyncInfo(
                    on_wait=keep, on_update=list(si.on_update)
                )
            i += 1
    if verbose:
        print(f"waitstrip: split {n_split} waits onto nops")
    return n_split


def drop_broken_range_clear(nc, verbose=False):
    """This walrus snapshot miscompiles EVENT_SEMAPHORE_RANGE_CLEAR ("ISA
    wrong length"). It only matters for re-executing an already-loaded NEFF
    with dirty semaphores; drop it (verified empirically with back-to-back
    executions)."""
    n = 0
    for bb_w in nc.m.functions[0].blocks:
        il = bb_w.instructions
        for i in range(len(il) - 1, -1, -1):
            ins = il[i]
            if type(ins).__name__ == "InstISA" and getattr(ins, "isa_opcode", 0) == 176:
                del il[i]
                n += 1
    if verbose:
        print(f"waitstrip: dropped {n} EVENT_SEMAPHORE_RANGE_CLEAR")


def legalize_waits(nc, verbose=False):
    drop_broken_range_clear(nc, verbose=verbose)
    strip_redundant_waits(nc, verbose=verbose)
    split_multi_waits(nc, verbose=verbose)
    bad = []
    for bb_w in nc.m.functions[0].blocks:
        for ins in bb_w.instructions:
            si = ins.sync_info
            if si is not None and len(si.on_wait) > 1:
                bad.append(ins.name)
    assert not bad, f"instructions still multi-wait: {bad}"


N = 8192
NC = 8
SH = N // NC          # 1024 rows per core
LR = SH + 2           # local rows incl halos = 1026
ALPHA = 0.1
LAM = 0.1
GAMMA = 0.001

BW = 2304             # band width (covers |i-j| <= 1088 for every tile row)
PAD = 1152            # zero padding on each side of A1c_pad
APW = N + 2 * PAD     # 10496
CB = PAD              # first real column inside A1c_pad
R0S = [126 * k for k in range(8)] + [LR - 128]   # tile starts (local rows)
NT = len(R0S)

f32 = mybir.dt.float32
bf16 = mybir.dt.bfloat16
i32 = mybir.dt.int32
Alu = mybir.AluOpType
Act = mybir.ActivationFunctionType
X = mybir.AxisListType.X

# accumulator column layout in the [128, 96] f32 output
SM_COL = 0     # 36 cols: tile k quarter q -> 4k+q, rows 0..125 (no host scale)
A2A_COL = 36   # 9 cols: DVE half of sum A1c^2 (host * dinv^2)
A2B_COL = 45   # 9 cols: ACT half of sum A1c^2 (host * dinv^2)
BD_COL = 54    # 9 cols: band sum A1c*decay (host * dinv)
DA_COL = 63    # 9 cols: ACT half of row sums d
DB_COL = 72    # 9 cols: DVE half of row sums d
ACC_W = 96


def _build_nc():
    nc = bass.Bass(num_devices=NC)
    adj_in = nc.dram_tensor("adj_sh", [LR, N], bf16, kind="ExternalInput")
    res_out = nc.dram_tensor("res", [128, ACC_W], f32, kind="ExternalOutput")

    with tile.TileContext(nc) as tc:
        with (
            tc.tile_pool(name="const", bufs=1) as cp,
            tc.tile_pool(name="dram", bufs=1, space="DRAM") as dram,
            tc.tile_pool(name="io", bufs=3) as iop,
            tc.tile_pool(name="apad", bufs=1) as apadp,
            tc.tile_pool(name="lr", bufs=1) as lrp,
            tc.tile_pool(name="scr", bufs=1) as scrp,
            tc.tile_pool(name="ps", bufs=1, space="PSUM") as psp,
        ):
            acc = cp.tile([128, ACC_W], f32)
            nc.vector.memset(acc[:], 0.0)
            epsb = cp.tile([128, 1], f32)
            nc.vector.memset(epsb[:], 1e-10)

            apads = [
                apadp.tile([128, APW], bf16, tag=f"apad{i}", name=f"apad{i}")
                for i in range(2)
            ]
            for a_t in apads:
                nc.vector.memset(a_t[:, 0:PAD], 0.0)
                nc.vector.memset(a_t[:, PAD + N : APW], 0.0)
            lrs = [
                lrp.tile([128, N], bf16, tag=f"lr{i}", name=f"lr{i}") for i in range(2)
            ]
            psums = [
                psp.tile([128, 2048], f32, tag=f"ps{i}", name=f"ps{i}")
                for i in range(2)
            ]

            # ---- stencil lhsT matrices: Mv[p,l]: 1.0 at p==l+1, -0.25 at p==l, l+2
            #      NI[p,l]: -0.25 at p==l+1
            Mv = cp.tile([128, 126], bf16)
            NI = cp.tile([128, 126], bf16)
            idx = cp.tile([128, 126], i32)
            nc.gpsimd.iota(idx[:], pattern=[[-1, 126]], base=0, channel_multiplier=1)
            idxf = cp.tile([128, 126], f32)
            nc.gpsimd.tensor_copy(idxf[:], idx[:])
            vm1 = cp.tile([128, 126], f32)
            nc.vector.tensor_scalar(vm1[:], idxf[:], 1.0, None, Alu.subtract)  # p-l-1
            vab = cp.tile([128, 126], f32)
            vneg = cp.tile([128, 126], f32)
            nc.vector.tensor_scalar(vneg[:], vm1[:], -1.0, None, Alu.mult)
            nc.vector.tensor_max(vab[:], vm1[:], vneg[:])                      # |p-l-1|
            near = cp.tile([128, 126], f32)
            nc.vector.tensor_scalar(near[:], vab[:], 1.0, None, Alu.is_le)     # |.|<=1
            ctr = cp.tile([128, 126], f32)
            nc.vector.tensor_scalar(ctr[:], vab[:], 0.0, None, Alu.is_equal)   # ==0
            near4 = cp.tile([128, 126], f32)
            nc.vector.tensor_scalar(near4[:], near[:], 0.25, None, Alu.mult)
            ctr125 = cp.tile([128, 126], f32)
            nc.vector.tensor_scalar(ctr125[:], ctr[:], 1.25, None, Alu.mult)
            nc.vector.tensor_sub(Mv[:], ctr125[:], near4[:])
            nc.vector.tensor_scalar(NI[:], ctr[:], -0.25, None, Alu.mult)

            # ---- identity for the PE transpose of d
            iden_i = cp.tile([128, 128], i32)
            nc.gpsimd.iota(
                iden_i[:], pattern=[[-1, 128]], base=0, channel_multiplier=1
            )
            idnf = cp.tile([128, 128], f32)
            nc.gpsimd.tensor_copy(idnf[:], iden_i[:])
            idn0 = cp.tile([128, 128], f32)
            nc.vector.tensor_scalar(idn0[:], idnf[:], 0.0, None, Alu.is_equal)
            idn = cp.tile([128, 128], bf16)
            nc.vector.tensor_copy(idn[:], idn0[:])

            # ---- decay band constant: D[p,u] = exp(-0.1*|1088 + p - u|)
            decayb = cp.tile([128, BW], bf16)
            bidx = scrp.tile([128, BW], i32, tag="djunk", name="bidx")
            nc.gpsimd.iota(bidx[:], pattern=[[-1, BW]], base=1088, channel_multiplier=1)
            bidf = scrp.tile([128, BW], f32, tag="sabs", name="bidf")
            nc.gpsimd.tensor_copy(bidf[:], bidx[:])
            babs = scrp.tile([128, BW], f32, tag="djunk", name="babs")
            nc.scalar.activation(babs[:], bidf[:], Act.Abs)
            nc.scalar.activation(decayb[:], babs[:], Act.Exp, scale=-ALPHA)

            # ---- pass 1: d = row sums over all 1026 local rows (adj is relu'd
            # host-side). Split ACT Copy(accum_out) / DVE reduce halves.
            # Overlap rows get identical full-row sums; host masks dedupe.
            d_a = cp.tile([128, 16], f32)
            nc.vector.memset(d_a[:], 0.0)
            d_b = cp.tile([128, 16], f32)
            nc.vector.memset(d_b[:], 0.0)
            p1dst = scrp.tile([128, 4608], bf16, tag="ajunk", name="p1dst")
            for k, r0 in enumerate(R0S):
                t = iop.tile([128, N], bf16, tag="adj", name=f"p1t{k}")
                eng = nc.gpsimd if k % 2 else nc.sync
                eng.dma_start(t[:], adj_in[r0 : r0 + 128, :])
                nc.scalar.activation(
                    p1dst[:, 0:4608], t[:, 0:4608], Act.Copy,
                    accum_out=d_a[:, k : k + 1],
                )
                nc.vector.tensor_reduce(
                    d_b[:, k : k + 1], t[:, 4608:N], axis=X, op=Alu.add
                )
            d_tot = cp.tile([128, 16], f32)
            nc.vector.tensor_add(d_tot[:], d_a[:], d_b[:])

            # ---- AllGather of own d (local rows 1..1024 = global shard rows).
            # d is transposed to row-major via a PE matmul against the identity
            # (partition-strided 4-byte DMAs are ~8us each; this is 2 DMAs).
            d_totb = cp.tile([128, 16], bf16)
            nc.vector.tensor_copy(d_totb[:], d_tot[:])
            nc.tensor.matmul(
                psums[0][0:16, 0:128], d_totb[:], idn[:], start=True, stop=True
            )
            dT = cp.tile([16, 128], f32)
            nc.vector.tensor_copy(dT[:], psums[0][0:16, 0:128])
            dcore = dram.tile([1, SH], f32)
            nc.scalar.dma_start(
                dcore[0:1, 0:1008].rearrange("o (k p) -> (o k) p", k=8),
                dT[0:8, 1:127],
            )
            nc.scalar.dma_start(dcore[0:1, 1008:1024], dT[8:9, 111:127])
            dglob = dram.tile([NC, SH], f32)
            nc.gpsimd.collective_compute(
                "AllGather",
                Alu.bypass,
                replica_groups=[list(range(NC))],
                ins=[dcore.opt()],
                outs=[dglob.opt()],
            )

            # (overlaps the collective) local dinv for the stencil lhsT
            lnd = cp.tile([128, 16], f32)
            nc.scalar.activation(lnd[:, 0:9], d_tot[:, 0:9], Act.Ln, bias=epsb[:])
            dinv_sb = cp.tile([128, 16], f32)
            nc.scalar.activation(dinv_sb[:, 0:9], lnd[:, 0:9], Act.Exp, scale=-0.5)
            # d halves into the output accumulator for the host
            nc.vector.tensor_copy(acc[:, DA_COL : DA_COL + 9], d_a[:, 0:9])
            nc.vector.tensor_copy(acc[:, DB_COL : DB_COL + 9], d_b[:, 0:9])

            # ---- global column factors -> bf16 colfac tile [128, N]
            dg = cp.tile([128, 64], f32)
            nc.scalar.dma_start(
                dg[:],
                dglob[:].rearrange("a b -> (a b)").rearrange("(p t) -> p t", p=128),
            )
            lng = cp.tile([128, 64], f32)
            nc.scalar.activation(lng[:], dg[:], Act.Ln, bias=epsb[:])
            dgi = cp.tile([128, 64], f32)
            nc.scalar.activation(dgi[:], lng[:], Act.Exp, scale=-0.5)
            dgib = cp.tile([128, 64], bf16)
            nc.vector.tensor_copy(dgib[:], dgi[:])
            dinv1 = dram.tile([1, N], bf16)
            nc.scalar.dma_start(
                dinv1[0:1, :].rearrange("o (p t) -> (o p) t", p=128), dgib[:]
            )
            colfac = cp.tile([128, N], bf16)
            nc.sync.dma_start(
                colfac[:, 0 : N // 2], dinv1[0:1, 0 : N // 2].to_broadcast((128, N // 2))
            )
            nc.scalar.dma_start(
                colfac[:, N // 2 : N],
                dinv1[0:1, N // 2 : N].to_broadcast((128, N // 2)),
            )

            # ---- pass 2 (software-pipelined: tile k+1's A-build is emitted on
            # the in-order DVE queue BEFORE tile k's accumulation ops, so the
            # next tile's matmuls are never stuck behind DVE work that waits
            # on PE; psum edge zeroing runs on ACT, whose sabs read waits for
            # the same matmul group anyway)
            pid = nc.vector.partition_id()
            zcol = cp.tile([128, 1], f32)
            nc.vector.memset(zcol[:], 0.0)

            adj_ts = [None] * NT
            Mvks = [None] * NT
            NIks = [None] * NT

            def emit_dma(k):
                adj_ts[k] = iop.tile([128, N], bf16, tag="adj", name=f"adj{k}")
                eng = nc.gpsimd if k % 2 else nc.sync
                eng.dma_start(adj_ts[k][:], adj_in[R0S[k] : R0S[k] + 128, :])

            def emit_abuild(k):
                Apad = apads[k % 2]
                # A1c = adj * colfac (column factors only; row factor folded
                # into lhsT / host scaling)
                nc.vector.tensor_tensor(
                    Apad[:, CB : CB + N], adj_ts[k][:], colfac[:], Alu.mult
                )
                # lr = shift_left(A1c) + shift_right(A1c)
                nc.vector.tensor_tensor(
                    lrs[k % 2][:], Apad[:, CB - 1 : CB - 1 + N],
                    Apad[:, CB + 1 : CB + 1 + N], Alu.add
                )
                # row-scaled lhsT
                Mvks[k] = scrp.tile([128, 126], bf16, tag=f"mvk{k%2}", name=f"mvk{k}")
                NIks[k] = scrp.tile([128, 126], bf16, tag=f"nik{k%2}", name=f"nik{k}")
                nc.vector.tensor_scalar(
                    Mvks[k][:], Mv[:], dinv_sb[:, k : k + 1], None, Alu.mult
                )
                nc.vector.tensor_scalar(
                    NIks[k][:], NI[:], dinv_sb[:, k : k + 1], None, Alu.mult
                )

            def emit_accums(k):
                Apad = apads[k % 2]
                # DVE half of sum A1c^2 (row partials; host scales by dinv^2)
                sq = scrp.tile([128, 2944], bf16, tag="djunk", name=f"sq{k}")
                nc.vector.scalar_tensor_tensor(
                    sq[:],
                    Apad[:, CB : CB + 2944],
                    1.0,
                    Apad[:, CB : CB + 2944],
                    Alu.bypass,
                    Alu.mult,
                    accum_out=acc[:, A2A_COL + k : A2A_COL + k + 1],
                )
                # band sum A1c*decay (row partials; host scales by dinv)
                bscr = scrp.tile([128, BW], bf16, tag="djunk", name=f"bscr{k}")
                nc.vector.scalar_tensor_tensor(
                    bscr[:],
                    Apad[:, bass.ds(pid * SH + (R0S[k] + 63), BW)],
                    1.0,
                    decayb[:],
                    Alu.bypass,
                    Alu.mult,
                    accum_out=acc[:, BD_COL + k : BD_COL + k + 1],
                )

            emit_dma(0)
            emit_dma(1)
            # tile-0 A-build split in halves gated on the two colfac broadcast
            # halves, so the first stencil matmuls start as early as possible
            # after the collective
            Apad0, lr0 = apads[0], lrs[0]
            nc.vector.tensor_tensor(
                Apad0[:, CB : CB + N // 2], adj_ts[0][:, 0 : N // 2],
                colfac[:, 0 : N // 2], Alu.mult,
            )
            nc.vector.tensor_tensor(
                Apad0[:, CB + N // 2 : CB + N], adj_ts[0][:, N // 2 : N],
                colfac[:, N // 2 : N], Alu.mult,
            )
            nc.vector.tensor_tensor(
                lr0[:, 0:4064], Apad0[:, CB - 1 : CB + 4063],
                Apad0[:, CB + 1 : CB + 4065], Alu.add,
            )
            nc.vector.tensor_tensor(
                lr0[:, 4064:N], Apad0[:, CB + 4063 : CB + N - 1],
                Apad0[:, CB + 4065 : CB + N + 1], Alu.add,
            )
            Mvks[0] = scrp.tile([128, 126], bf16, tag="mvk0", name="mvk0")
            NIks[0] = scrp.tile([128, 126], bf16, tag="nik0", name="nik0")
            nc.vector.tensor_scalar(
                Mvks[0][:], Mv[:], dinv_sb[:, 0:1], None, Alu.mult
            )
            nc.vector.tensor_scalar(
                NIks[0][:], NI[:], dinv_sb[:, 0:1], None, Alu.mult
            )
            for k, r0 in enumerate(R0S):
                Apad = apads[k % 2]
                lr = lrs[k % 2]
                # ACT half of sum A1c^2 first: it fills ACT's idle time while
                # PE runs this tile's matmuls (ACT's sabs reads wait on PE
                # groups regardless), and keeps it off the final-tile tail
                sqb = scrp.tile([128, 5248], bf16, tag="ajunk", name=f"sqb{k}")
                nc.scalar.activation(
                    sqb[:],
                    Apad[:, CB + 2944 : CB + N],
                    Act.Square,
                    accum_out=acc[:, A2B_COL + k : A2B_COL + k + 1],
                )
                if k == NT - 1:
                    # last tile: no next A-build to pipeline; run the DVE
                    # accums under the matmuls instead of after them
                    emit_accums(k)
                # stencil: t = A - 0.25*(up+down+left+right), 2 matmuls/chunk
                for q in range(4):
                    ps = psums[q % 2]
                    for cc in range(4):
                        c = 4 * q + cc
                        col = CB + 512 * c
                        out_ap = ps[0:126, 512 * cc : 512 * cc + 512]
                        nc.tensor.matmul(
                            out_ap, Mvks[k][:], Apad[:, col : col + 512],
                            start=True, stop=False,
                        )
                        nc.tensor.matmul(
                            out_ap, NIks[k][:], lr[:, 512 * c : 512 * c + 512],
                            start=False, stop=True,
                        )
                    if q == 0:
                        nc.scalar.activation(ps[0:126, 0:1], zcol[0:126, :], Act.Copy)
                    if q == 3:
                        nc.scalar.activation(
                            ps[0:126, 2047:2048], zcol[0:126, :], Act.Copy
                        )
                    sabs = scrp.tile([126, 2048], bf16, tag="sabs", name=f"sabs{k}_{q}")
                    nc.scalar.activation(
                        sabs[:], ps[0:126, :], Act.Abs,
                        accum_out=acc[0:126, SM_COL + 4 * k + q : SM_COL + 4 * k + q + 1],
                    )

                if k + 2 < NT:
                    emit_dma(k + 2)
                if k + 1 < NT:
                    emit_abuild(k + 1)
                if k < NT - 1:
                    emit_accums(k)

            acc2 = cp.tile([128, ACC_W], f32)
            nc.vector.tensor_copy(acc2[:], acc[:])
            nc.sync.dma_start(res_out[:], acc2[:])

    legalize_waits(nc)
    nc.finalize()
    drop_broken_range_clear(nc)
    return nc


def _masks():
    """Row-ownership masks resolving overlap-tile double counting (per core),
    plus global row index per (core, partition, tile)."""
    sm = np.zeros((NC, 128, 36), np.float64)
    rows = np.zeros((NC, 128, 9), np.float64)
    grow = np.zeros((NC, 128, 9), np.int64)
    for c in range(NC):
        claimed_r = set()
        claimed_s = set()
        for k, r0 in enumerate(R0S):
            for p in range(128):
                L = r0 + p
                g = SH * c - 1 + L
                grow[c, p, k] = min(max(g, 0), N - 1)
                if 1 <= L <= 1024 and L not in claimed_r:
                    claimed_r.add(L)
                    rows[c, p, k] = 1.0
            for p in range(126):
                L = r0 + 1 + p           # stencil out row (local)
                g = SH * c - 1 + L       # global row
                if 1 <= L <= 1024 and 1 <= g <= N - 2 and L not in claimed_s:
                    claimed_s.add(L)
                    sm[c, p, 4 * k : 4 * k + 4] = 1.0
    return sm, rows, grow


_SM_MASK, _ROW_MASK, _GROW = _masks()


def _analytic_decay_sq():
    k = np.arange(1, N, dtype=np.float64)
    return N + 2.0 * np.sum((N - k) * np.exp(-2.0 * ALPHA * k))


_NC_CACHE = None


def _prepare_in_maps(adj):
    import ml_dtypes

    in_maps = []
    for c in range(NC):
        sl = np.zeros((LR, N), ml_dtypes.bfloat16)
        lo = SH * c - 1
        src_lo = max(lo, 0)
        src_hi = min(lo + LR, N)
        sl[src_lo - lo : src_hi - lo, :] = np.maximum(adj[src_lo:src_hi], 0).astype(
            ml_dtypes.bfloat16
        )
        in_maps.append({"adj_sh": sl})
    return in_maps


def _reduce(results):
    # reconstruct per-global-row degree from the returned halves
    d_g = np.zeros(N, np.float64)
    parts = []
    for c in range(NC):
        o = results[c]["res"].astype(np.float64)
        parts.append(o)
        d = o[:, DA_COL : DA_COL + 9] + o[:, DB_COL : DB_COL + 9]
        np.add.at(d_g, _GROW[c], _ROW_MASK[c] * d)
    dinv_g = 1.0 / np.sqrt(d_g + 1e-10)

    s_sm = 0.0
    s_a2 = 0.0
    s_bd = 0.0
    for c in range(NC):
        o = parts[c]
        dv = dinv_g[_GROW[c]]                      # [128, 9]
        s_sm += float((o[:, SM_COL : SM_COL + 36] * _SM_MASK[c]).sum())
        a2 = o[:, A2A_COL : A2A_COL + 9] + o[:, A2B_COL : A2B_COL + 9]
        s_a2 += float((a2 * _ROW_MASK[c] * dv * dv).sum())
        s_bd += float((o[:, BD_COL : BD_COL + 9] * _ROW_MASK[c] * dv).sum())

    d2 = _analytic_decay_sq()
    loss = (s_a2 - 2.0 * s_bd + d2) + LAM * s_sm + GAMMA * s_a2
    return np.array(loss, dtype=np.float32)


def kernel(adj):
    global _NC_CACHE
    adj = np.ascontiguousarray(np.asarray(adj), dtype=np.float32)
    assert adj.shape == (N, N)

    if _NC_CACHE is None:
        _NC_CACHE = _build_nc()
    nc = _NC_CACHE

    res = run_bass_kernel_spmd(nc, _prepare_in_maps(adj), core_ids=list(range(NC)))
    return _reduce(res.results)
